# revision 4
# baseline (speedup 1.0000x reference)
"""Trainium2 Bass kernel for the DFS-Mixer style-attention module.

Computation (per batch b):
    dot[k,c]  = sum_hw CT[c,hw] * CR[k,c,hw]
    norm[k,c] = sqrt(sum_hw CR[k,c,hw]^2)
    w[.,c]    = softmax_k(2 * dot[.,c] / norm[.,c])
    out[c,hw] = sum_k IR[k,c,hw] * w[k,c]

Sharding: data-parallel over batch B=8 across the 8 NeuronCores (one b per
core, no cross-core communication).

Per-core layout: C=256 is tiled as 2 x 128 SBUF partitions, HW=64*64=4096 on
the free axis.  Phase 1 uses the DVE fused multiply+reduce
(tensor_tensor_reduce) for the dots and the ACT Square-with-accumulate for the
norms.  Phase 3 runs on the otherwise idle TensorEngine: scaling by a
per-partition scalar w[k,c] is a matmul with the 128x128 diagonal matrix
diag(w[:,k]) as the stationary operand, and the sum over k accumulates in
PSUM for free.
"""

import os
import sys

import numpy as np


def _import_concourse():
    try:
        import concourse.bass  # noqa: F401
    except ImportError:
        for p in ("/opt/trn_rl_repo", "/root/.axon_site/_ro/trn_rl_repo"):
            if os.path.isdir(p) and p not in sys.path:
                sys.path.insert(0, p)
        import concourse.bass  # noqa: F401


_import_concourse()

import concourse.bass as bass  # noqa: E402
import concourse.mybir as mybir  # noqa: E402
from concourse import tile  # noqa: E402
from concourse.bass_utils import run_bass_kernel_spmd  # noqa: E402
from concourse.vector_clock import ScopedClock, VectorClock  # noqa: E402


def _split_multiwait_bir(bir: bytes) -> bytes:
    """The neuronxcc walrus in this container encodes at most ONE sync-wait
    per instruction; Tile emits several.  Hoist extra waits onto same-engine
    NoOp instructions inserted immediately before the original instruction
    (engines execute in order, so waiting earlier on the same engine is
    semantically identical).  Sem *updates* are left untouched (a DMA's
    completion-inc cannot move to a sequencer NoOp)."""
    import json

    j = json.loads(bir)
    ctr = 0
    for f in j.get("functions", []):
        for bb in f.get("blocks", []):
            out_insts = []
            for ins in bb.get("instructions", []):
                si = ins.get("sync_info")
                waits = (si or {}).get("on_wait") or []
                if len(waits) > 1:
                    for w in waits[:-1]:
                        ctr += 1
                        nop = {
                            "engine": ins["engine"],
                            "ins": [],
                            "outs": [],
                            "name": f"waitsplit-{ctr}",
                            "opcode": "NoOp",
                            "sync_info": {"on_update": [], "on_wait": [w]},
                        }
                        if "debug" in ins:
                            nop["debug"] = ins["debug"]
                        out_insts.append(nop)
                    si["on_wait"] = [waits[-1]]
                out_insts.append(ins)
            bb["instructions"] = out_insts
    return json.dumps(j).encode()


_orig_to_json_bytes = bass.Bass.to_json_bytes


def _patched_to_json_bytes(self, *a, **kw):
    return _split_multiwait_bir(_orig_to_json_bytes(self, *a, **kw))


bass.Bass.to_json_bytes = _patched_to_json_bytes


def _patched_drain_and_barrier(self, tick_clock, wait_clock):
    # The walrus build in this container encodes at most one sync-wait per
    # SP CTRL instruction; TileContext's stock exit emits a single Drain
    # waiting on every used semaphore and fails codegen ("Too many sync
    # wait commands").  Split into one Drain per semaphore instead.
    gc = tick_clock.global_clock
    n = len(gc)
    nonzero = [p for p in range(n) if gc[p] > 0] or [0]
    for p in nonzero:
        d = self.nc.sync.drain()
        vec = [gc[q] if q == p else 0 for q in range(n)]
        wait_clock.add_sem_waits(d.ins, ScopedClock({None: VectorClock(vec)}))
    self.nc.all_engine_barrier()
    popped = self.nc._tile_sem_poison_stack.pop()
    assert popped is self._sem_poison
    self.nc.clear_and_free_semaphores(list(self.sems.allocated().values()))
    self.nc.all_engine_barrier()


tile.TileContext._drain_and_barrier = _patched_drain_and_barrier

FP = mybir.dt.float32
B, K, C, H, W = 8, 8, 256, 64, 64
HW = H * W
P = 128                 # SBUF partitions
NCT = C // P            # 2 c-tiles per core
MMN = 512               # moving free dim per matmul (= one PSUM bank of f32)
NMM = HW // MMN         # 8 matmuls per (k, c-tile)

_AF = mybir.ActivationFunctionType
_OP = mybir.AluOpType


def build_nc() -> bass.Bass:
    nc = bass.Bass()
    IR = nc.declare_dram_parameter("IR", [K, C, HW], FP, isOutput=False)
    CR = nc.declare_dram_parameter("CR", [K, C, HW], FP, isOutput=False)
    CT = nc.declare_dram_parameter("CT", [C, HW], FP, isOutput=False)
    OUT = nc.declare_dram_parameter("OUT", [C, HW], FP, isOutput=True)

    with tile.TileContext(nc) as tc:
        with (
            tc.tile_pool(name="ctp", bufs=1) as ct_pool,
            tc.tile_pool(name="crp", bufs=2) as cr_pool,
            tc.tile_pool(name="irp", bufs=3) as ir_pool,
            tc.tile_pool(name="scr", bufs=2) as scr_pool,
            tc.tile_pool(name="sml", bufs=1) as small,
            tc.tile_pool(name="wkp", bufs=2) as wk_pool,
            tc.tile_pool(name="obp", bufs=2) as out_pool,
            tc.tile_pool(name="psp", bufs=1, space="PSUM") as psum_pool,
        ):
            # Diagonal ones mask, built once: mask[p, f] = (p == f).
            ones_t = small.tile([P, P], FP, tag="ones")
            nc.vector.memset(ones_t[:], 1.0)
            mask = small.tile([P, P], FP, tag="mask")
            nc.gpsimd.affine_select(
                mask[:],
                ones_t[:],
                pattern=[[-1, P]],
                compare_op=_OP.is_equal,
                fill=0.0,
                base=0,
                channel_multiplier=1,
            )

            # Content-target features stay resident in SBUF (reused by all k).
            ct_tiles = []
            for t in range(NCT):
                ctt = ct_pool.tile([P, HW], FP, tag=f"ct{t}")
                nc.sync.dma_start(out=ctt[:], in_=CT[t * P:(t + 1) * P, :])
                ct_tiles.append(ctt)

            for t in range(NCT):
                cs = slice(t * P, (t + 1) * P)

                # ---- Phase 1: dot[c,k] and sq[c,k] reductions over HW ----
                dot = small.tile([P, K], FP, tag=f"dot{t}")
                sq = small.tile([P, K], FP, tag=f"sq{t}")
                for k in range(K):
                    crt = cr_pool.tile([P, HW], FP, tag="cr")
                    nc.sync.dma_start(out=crt[:], in_=CR[k, cs, :])
                    scr_d = scr_pool.tile([P, HW], FP, tag="scr")
                    nc.vector.tensor_mul(scr_d[:], ct_tiles[t][:], crt[:])
                    nc.vector.reduce_sum(
                        dot[:, k:k + 1], scr_d[:], axis=mybir.AxisListType.X
                    )
                    scr_s = scr_pool.tile([P, HW], FP, tag="scr")
                    nc.scalar.activation(
                        out=scr_s[:],
                        in_=crt[:],
                        func=_AF.Square,
                        accum_out=sq[:, k:k + 1],
                    )

                # ---- Softmax over k (tiny [128, 8] ops) ----
                norm = small.tile([P, K], FP, tag=f"norm{t}")
                nc.scalar.activation(norm[:], sq[:], func=_AF.Sqrt)
                rnorm = small.tile([P, K], FP, tag=f"rnorm{t}")
                nc.vector.reciprocal(rnorm[:], norm[:])
                sim = small.tile([P, K], FP, tag=f"sim{t}")
                nc.vector.tensor_mul(sim[:], dot[:], rnorm[:])
                mx = small.tile([P, 1], FP, tag=f"mx{t}")
                nc.vector.reduce_max(mx[:], sim[:], axis=mybir.AxisListType.X)
                nbias = small.tile([P, 1], FP, tag=f"nb{t}")
                nc.vector.tensor_scalar_mul(nbias[:], mx[:], -2.0)
                # e = exp(2*sim - 2*max)
                e = small.tile([P, K], FP, tag=f"e{t}")
                nc.scalar.activation(
                    e[:], sim[:], func=_AF.Exp, bias=nbias[:, 0:1], scale=2.0
                )
                s = small.tile([P, 1], FP, tag=f"s{t}")
                nc.vector.reduce_sum(s[:], e[:], axis=mybir.AxisListType.X)
                rs = small.tile([P, 1], FP, tag=f"rs{t}")
                nc.vector.reciprocal(rs[:], s[:])
                w = small.tile([P, K], FP, tag=f"w{t}")
                nc.vector.tensor_scalar_mul(w[:], e[:], rs[:, 0:1])

                # ---- Phase 3: out[c,:] = sum_k IR[k,c,:] * w[c,k] on PE ----
                acc = psum_pool.tile([P, HW], FP, tag="acc")
                for k in range(K):
                    irt = ir_pool.tile([P, HW], FP, tag="ir")
                    nc.sync.dma_start(out=irt[:], in_=IR[k, cs, :])
                    wm = wk_pool.tile([P, P], FP, tag="wm")
                    nc.vector.tensor_scalar_mul(wm[:], mask[:], w[:, k:k + 1])
                    for j in range(NMM):
                        nc.tensor.matmul(
                            acc[:, j * MMN:(j + 1) * MMN],
                            wm[:],
                            irt[:, j * MMN:(j + 1) * MMN],
                            start=(k == 0),
                            stop=(k == K - 1),
                        )

                ob = out_pool.tile([P, HW], FP, tag="ob")
                nc.scalar.copy(ob[:], acc[:])
                nc.sync.dma_start(out=OUT[cs, :], in_=ob[:])

    return nc


_NC_CACHE = None


def _get_nc() -> bass.Bass:
    global _NC_CACHE
    if _NC_CACHE is None:
        _NC_CACHE = build_nc()
    return _NC_CACHE


def run(inputs: dict, trace: bool = False):
    """Shard over B, run on 8 cores, gather. Returns (output, BassKernelResults)."""
    ir = np.ascontiguousarray(np.asarray(inputs["IR_features"], dtype=np.float32))
    cr = np.ascontiguousarray(np.asarray(inputs["CR_features"], dtype=np.float32))
    ct = np.ascontiguousarray(np.asarray(inputs["CT_feature"], dtype=np.float32))
    assert ir.shape == (B, K, C, H, W) and cr.shape == (B, K, C, H, W)
    assert ct.shape == (B, C, H, W)

    in_maps = [
        {
            "IR": ir[b].reshape(K, C, HW),
            "CR": cr[b].reshape(K, C, HW),
            "CT": ct[b].reshape(C, HW),
        }
        for b in range(B)
    ]
    res = run_bass_kernel_spmd(_get_nc(), in_maps, list(range(B)), trace=trace)
    out = np.stack([res.results[b]["OUT"] for b in range(B)])
    return out.reshape(B, C, H, W).astype(np.float32), res


def kernel(**inputs) -> np.ndarray:
    return run(inputs)[0]
